# revision 1
# baseline (speedup 1.0000x reference)
"""Trainium2 Bass kernel for nn_GroupedConvFuseSide4.

out[b,k] = w[k,0]*side5[b,k] + w[k,1]*side4[b,k]
         + w[k,2]*side1[b,0] + w[k,3]*side2[b,0] + w[k,4]*side3[b,0] + bias[k]

Sharding: pure data parallel over batch (B=8) across 8 NeuronCores.

Per-core scheme ("packed partitions", host-repacked): the 262144 pixels of
one batch image are split into 128 chunks of 2048. A tile covers G=6 chunks
x all 19 channels on partitions p = 19*g + k (114 partitions, free 2048):
  - PE matmul (contraction 19 = ones row + [s1,s2,s3] x 6 groups, float32r
    at 1 cycle/row) computes base = w2*s1 + w3*s2 + w4*s3 + bias for all
    114 partitions into PSUM.
  - DVE merges side5/side4 with two scalar_tensor_tensor ops using
    per-partition weight vectors.
All tensors are repacked on the host into the tile layout so every DMA is
a contiguous [rows, 8KB] block (full 16-engine DMA fanout). Weights/bias
are baked into the program (inline const tensors / matmul weights).
"""

import numpy as np

B, K, H, W = 8, 19, 512, 512
CH = 128                   # chunks per image
FD = 2048                  # elems per chunk
G = 6                      # chunk-groups per full tile
NT = 21                    # full tiles (126 chunks); tail tile has G=2
PT = 19 * G                # 114 partitions in a full tile
N_CORES = 8

_cache = {}


def _build_program(w, b):
    import concourse.bacc as bacc
    import concourse.tile as tile
    import concourse.mybir as mybir
    from contextlib import ExitStack

    f32 = mybir.dt.float32
    f32r = mybir.dt.float32r
    mult = mybir.AluOpType.mult
    add = mybir.AluOpType.add

    nc = bacc.Bacc(
        "TRN2", target_bir_lowering=False, debug=False,
        enable_asserts=False, num_devices=N_CORES,
    )

    x5a = nc.dram_tensor("x5a", [NT, PT, FD], f32, kind="ExternalInput").ap()
    x5b = nc.dram_tensor("x5b", [38, FD], f32, kind="ExternalInput").ap()
    x4a = nc.dram_tensor("x4a", [NT, PT, FD], f32, kind="ExternalInput").ap()
    x4b = nc.dram_tensor("x4b", [38, FD], f32, kind="ExternalInput").ap()
    xsa = nc.dram_tensor("xsa", [NT, 3 * G, FD], f32, kind="ExternalInput").ap()
    xsb = nc.dram_tensor("xsb", [6, FD], f32, kind="ExternalInput").ap()
    outa = nc.dram_tensor("outa", [NT, PT, FD], f32, kind="ExternalOutput").ap()
    outb = nc.dram_tensor("outb", [38, FD], f32, kind="ExternalOutput").ap()

    # ---- baked constants ----
    def wvec(col, g):
        return np.tile(w[:, col], g).reshape(-1, 1).astype(np.float32)

    # lhsT: [1 + 3*g_cnt contraction, 19*g_cnt out]; row 0 = ones row
    # carrying the bias; row 1 + g_cnt*s + g = single s, group g.
    def make_lhsT(g_cnt):
        rows = 3 * g_cnt + 1
        m = np.zeros((rows, 19 * g_cnt), dtype=np.float32)
        for g in range(g_cnt):
            for k in range(K):
                p = 19 * g + k
                m[0, p] = b[k]
                m[1 + g_cnt * 0 + g, p] = w[k, 2]
                m[1 + g_cnt * 1 + g, p] = w[k, 3]
                m[1 + g_cnt * 2 + g, p] = w[k, 4]
        return m

    w0_d = nc.inline_tensor(wvec(0, G), name="w0vec").ap()
    w1_d = nc.inline_tensor(wvec(1, G), name="w1vec").ap()
    lhsT_d = nc.inline_tensor(make_lhsT(G), name="lhsT6").ap()
    lhsT2_d = nc.inline_tensor(make_lhsT(2), name="lhsT2").ap()

    XR = 3 * G + 1         # 19 rows in the singles+ones tile

    with tile.TileContext(nc) as tc, ExitStack() as ctx:
        consts = ctx.enter_context(tc.tile_pool(name="consts", bufs=1))
        xs_pool = ctx.enter_context(tc.tile_pool(name="xs", bufs=1))
        x5_pool = ctx.enter_context(tc.tile_pool(name="x5", bufs=4))
        x4_pool = ctx.enter_context(tc.tile_pool(name="x4", bufs=4))
        d_pool = ctx.enter_context(tc.tile_pool(name="d", bufs=3))
        o_pool = ctx.enter_context(tc.tile_pool(name="o", bufs=4))
        psum_pool = ctx.enter_context(tc.tile_pool(name="ps", bufs=2, space="PSUM"))

        w0t = consts.tile([PT, 1], f32, tag="w0")
        w1t = consts.tile([PT, 1], f32, tag="w1")
        lt6 = consts.tile([XR, PT], f32, tag="lt6")
        lt2 = consts.tile([7, 38], f32, tag="lt2")
        nc.sync.dma_start(out=w0t[:], in_=w0_d)
        nc.sync.dma_start(out=w1t[:], in_=w1_d)
        nc.sync.dma_start(out=lt6[:], in_=lhsT_d)
        nc.sync.dma_start(out=lt2[:], in_=lhsT2_d)

        # persistent singles tiles (ring of 3); ones row 0 memset once each
        n_xs = 3
        xs_tiles = []
        for i in range(n_xs):
            xs = xs_pool.tile([XR, FD], f32, tag=f"xs{i}")
            nc.vector.memset(xs[0:1, :], 1.0)
            xs_tiles.append(xs)
        xs2 = xs_pool.tile([7, FD], f32, tag="xs2")
        nc.vector.memset(xs2[0:1, :], 1.0)

        def split_dma(eng, dst_fn, src_fn, rows):
            # 114-row DMAs fan out to only 6 of 16 SDMA engines; any count
            # <= 112 fans out to all 16, so split at 64.
            if rows > 112:
                eng.dma_start(out=dst_fn(0, 64), in_=src_fn(0, 64))
                eng.dma_start(out=dst_fn(64, rows), in_=src_fn(64, rows))
            else:
                eng.dma_start(out=dst_fn(0, rows), in_=src_fn(0, rows))

        def do_tile(x5_src, x4_src, xs_src, out_dst, g_cnt, xs, lt):
            pt = 19 * g_cnt

            x5 = x5_pool.tile([PT, FD], f32, tag="x5")
            split_dma(nc.sync, lambda a, z: x5[a:z, :], lambda a, z: x5_src[a:z], pt)
            x4 = x4_pool.tile([PT, FD], f32, tag="x4")
            split_dma(nc.scalar, lambda a, z: x4[a:z, :], lambda a, z: x4_src[a:z], pt)
            nc.scalar.dma_start(out=xs[1:1 + 3 * g_cnt, :], in_=xs_src)

            ps = psum_pool.tile([PT, FD], f32, tag="ps")
            for i in range(FD // 512):
                nc.tensor.matmul(
                    ps[:pt, 512 * i:512 * (i + 1)],
                    lt[:],
                    xs[:, 512 * i:512 * (i + 1)],
                    start=True, stop=True,
                )

            d = d_pool.tile([PT, FD], f32, tag="d")
            nc.vector.scalar_tensor_tensor(
                d[:pt, :], x5[:pt, :], w0t[:pt, :], ps[:pt, :], mult, add)
            o = o_pool.tile([PT, FD], f32, tag="o")
            nc.vector.scalar_tensor_tensor(
                o[:pt, :], x4[:pt, :], w1t[:pt, :], d[:pt, :], mult, add)

            split_dma(nc.sync, lambda a, z: out_dst[a:z], lambda a, z: o[a:z, :], pt)

        for t in range(NT):
            do_tile(x5a[t], x4a[t], xsa[t], outa[t], G,
                    xs_tiles[t % n_xs], lt6)
        do_tile(x5b, x4b, xsb, outb, 2, xs2, lt2)

    nc.compile()
    return nc


def _get_program(w, b):
    key = (w.tobytes(), b.tobytes())
    if key not in _cache:
        _cache[key] = _build_program(w, b)
    return _cache[key]


def _pack_kchw(a):
    """[K, CH, FD] -> main [NT, PT, FD] (p = 19g+k), tail [38, FD]."""
    main = a[:, :G * NT].reshape(K, NT, G, FD).transpose(1, 2, 0, 3).reshape(NT, PT, FD)
    tail = a[:, G * NT:].transpose(1, 0, 2).reshape(2 * K, FD)
    return np.ascontiguousarray(main), np.ascontiguousarray(tail)


def _unpack_out(main, tail):
    """inverse of _pack_kchw -> [K, CH, FD]"""
    a = main.reshape(NT, G, K, FD).transpose(2, 0, 1, 3).reshape(K, G * NT, FD)
    b_ = tail.reshape(2, K, FD).transpose(1, 0, 2)
    return np.concatenate([a, b_], axis=1)


def run(inputs, trace=False, tmpdir=None):
    from concourse.bass_utils import run_bass_kernel_spmd

    w = np.asarray(inputs["weight"], dtype=np.float32)
    b = np.asarray(inputs["bias"], dtype=np.float32)
    nc = _get_program(w, b)

    s1f = np.asarray(inputs["side1"]).reshape(B, CH, FD)
    s2f = np.asarray(inputs["side2"]).reshape(B, CH, FD)
    s3f = np.asarray(inputs["side3"]).reshape(B, CH, FD)
    s4f = np.asarray(inputs["side4"]).reshape(B, K, CH, FD)
    s5f = np.asarray(inputs["side5"]).reshape(B, K, CH, FD)

    in_maps = []
    for c in range(N_CORES):
        x5a, x5b = _pack_kchw(s5f[c])
        x4a, x4b = _pack_kchw(s4f[c])
        xsa = np.ascontiguousarray(np.concatenate(
            [s1f[c, :G * NT].reshape(NT, G, FD),
             s2f[c, :G * NT].reshape(NT, G, FD),
             s3f[c, :G * NT].reshape(NT, G, FD)], axis=1))
        xsb = np.ascontiguousarray(np.concatenate(
            [s1f[c, G * NT:], s2f[c, G * NT:], s3f[c, G * NT:]], axis=0))
        in_maps.append({
            "x5a": x5a, "x5b": x5b, "x4a": x4a, "x4b": x4b,
            "xsa": xsa, "xsb": xsb,
        })

    res = run_bass_kernel_spmd(nc, in_maps, list(range(N_CORES)),
                               trace=trace, tmpdir=tmpdir)
    outs = []
    for c in range(N_CORES):
        o = _unpack_out(res.results[c]["outa"], res.results[c]["outb"])
        outs.append(o.reshape(1, K, H, W))
    return np.concatenate(outs, axis=0), res


def kernel(**inputs):
    out, _ = run(inputs, trace=False)
    return out



# revision 2
# speedup vs baseline: 2.2596x; 2.2596x over previous
"""Trainium2 Bass kernel for nn_GroupedConvFuseSide4.

out[b,k] = w[k,0]*side5[b,k] + w[k,1]*side4[b,k]
         + w[k,2]*side1[b,0] + w[k,3]*side2[b,0] + w[k,4]*side3[b,0] + bias[k]

Sharding: pure data parallel over batch (B=8) across 8 NeuronCores.

Per-core scheme (fp16 staging, 128-partition packed pairs): the op is
memory-bound, so all large tensors are staged in DRAM as fp16 (host converts;
rel-err ~1e-3 vs the 2e-2 gate). The 262144 pixels of one image are split
into CH=32 chunks of FD=8192. The (chunk, k) pairs are enumerated
chunk-major into 608 rows; tiles take 128 consecutive rows (4 full tiles +
a 96-row tail), so every side5/side4/out DMA is one contiguous
[128, 16KB] = 2MB transfer with full 16-engine fanout.

Per tile: PE matmul (contraction = ones row + 3 singles x nct chunks, fp16)
computes base = w2*s1 + w3*s2 + w4*s3 + bias into fp32 PSUM; the scalar
engine (ACT) evacuates PSUM to fp16 SBUF; DVE merges side5/side4 with two
all-fp16 scalar_tensor_tensor ops (2x packed mode) using per-partition
weight vectors. Weights/bias/lhsT are baked into the program as inline
const tensors.
"""

import numpy as np

B, K, H, W = 8, 19, 512, 512
NPIX = H * W               # 262144 pixels per channel image
FD = 8192                  # pixels per chunk
CH = NPIX // FD            # 32 chunks per image
NPAIR = CH * K             # 608 (chunk, k) pairs, chunk-major: i -> (i//K, i%K)
PT = 128                   # partitions per full tile
NT = NPAIR // PT           # 4 full tiles
TAIL = NPAIR - NT * PT     # 96-row tail tile
N_CORES = 8

# per-tile geometry: (row offset, partitions, first chunk, n chunks)
_TILES = []
for _t in range(NT + 1):
    _i0 = _t * PT
    _pt = PT if _t < NT else TAIL
    _c0 = _i0 // K
    _c1 = (_i0 + _pt - 1) // K
    _TILES.append((_i0, _pt, _c0, _c1 - _c0 + 1))

_XS_OFF = []               # row offsets of each tile's singles block in xsall
_o = 0
for _, _, _, _nct in _TILES:
    _XS_OFF.append(_o)
    _o += 3 * _nct
XS_ROWS = _o               # 108

_cache = {}


def _build_program(w, b):
    import concourse.bacc as bacc
    import concourse.tile as tile
    import concourse.mybir as mybir
    from contextlib import ExitStack

    f16 = mybir.dt.float16
    f32 = mybir.dt.float32
    mult = mybir.AluOpType.mult
    add = mybir.AluOpType.add

    nc = bacc.Bacc(
        "TRN2", target_bir_lowering=False, debug=False,
        enable_asserts=False, num_devices=N_CORES,
    )

    x5d = nc.dram_tensor("x5", [NPAIR, FD], f16, kind="ExternalInput").ap()
    x4d = nc.dram_tensor("x4", [NPAIR, FD], f16, kind="ExternalInput").ap()
    xsd = nc.dram_tensor("xs", [XS_ROWS, FD], f16, kind="ExternalInput").ap()
    outd = nc.dram_tensor("out", [NPAIR, FD], f16, kind="ExternalOutput").ap()

    # ---- baked constants (per tile: lhsT + the two per-partition w vecs) ----
    def tile_consts(t):
        i0, pt, c0, nct = _TILES[t]
        rows = 1 + 3 * nct
        lhsT = np.zeros((rows, pt), dtype=np.float16)
        w0 = np.zeros((pt, 1), dtype=np.float16)
        w1 = np.zeros((pt, 1), dtype=np.float16)
        for p in range(pt):
            i = i0 + p
            ch, k = i // K, i % K
            j = ch - c0
            lhsT[0, p] = b[k]
            lhsT[1 + 0 * nct + j, p] = w[k, 2]
            lhsT[1 + 1 * nct + j, p] = w[k, 3]
            lhsT[1 + 2 * nct + j, p] = w[k, 4]
            w0[p, 0] = w[k, 0]
            w1[p, 0] = w[k, 1]
        return lhsT, w0, w1

    lhsT_d, w0_d, w1_d = [], [], []
    for t in range(NT + 1):
        lhsT, w0, w1 = tile_consts(t)
        lhsT_d.append(nc.inline_tensor(lhsT, name=f"lhsT{t}").ap())
        w0_d.append(nc.inline_tensor(w0, name=f"w0v{t}").ap())
        w1_d.append(nc.inline_tensor(w1, name=f"w1v{t}").ap())

    MAXROWS = 1 + 3 * max(nct for _, _, _, nct in _TILES)  # 25
    CC = 2048                  # compute chunk (4 PSUM banks fp32)

    with tile.TileContext(nc) as tc, ExitStack() as ctx:
        consts = ctx.enter_context(tc.tile_pool(name="consts", bufs=1))
        xs_pool = ctx.enter_context(tc.tile_pool(name="xs", bufs=1))
        x5_pool = ctx.enter_context(tc.tile_pool(name="x5", bufs=2))
        x4_pool = ctx.enter_context(tc.tile_pool(name="x4", bufs=2))
        b_pool = ctx.enter_context(tc.tile_pool(name="bb", bufs=3))
        d_pool = ctx.enter_context(tc.tile_pool(name="d", bufs=3))
        o_pool = ctx.enter_context(tc.tile_pool(name="o", bufs=2))
        psum_pool = ctx.enter_context(tc.tile_pool(name="ps", bufs=2, space="PSUM"))

        lts, w0ts, w1ts = [], [], []
        for t in range(NT + 1):
            _, pt, _, nct = _TILES[t]
            rows = 1 + 3 * nct
            lt = consts.tile([rows, pt], f16, tag=f"lt{t}")
            w0t = consts.tile([pt, 1], f16, tag=f"w0t{t}")
            w1t = consts.tile([pt, 1], f16, tag=f"w1t{t}")
            nc.sync.dma_start(out=lt[:], in_=lhsT_d[t])
            nc.sync.dma_start(out=w0t[:], in_=w0_d[t])
            nc.sync.dma_start(out=w1t[:], in_=w1_d[t])
            lts.append(lt)
            w0ts.append(w0t)
            w1ts.append(w1t)

        # persistent singles tiles (ring of 3); ones row 0 memset once each
        xs_tiles = []
        for i in range(3):
            xs = xs_pool.tile([MAXROWS, FD], f16, tag=f"xs{i}")
            nc.vector.memset(xs[0:1, :], 1.0)
            xs_tiles.append(xs)

        for t in range(NT + 1):
            i0, pt, c0, nct = _TILES[t]
            rows = 1 + 3 * nct
            xs = xs_tiles[t % 3]

            x5 = x5_pool.tile([PT, FD], f16, tag="x5")
            nc.sync.dma_start(out=x5[:pt, :], in_=x5d[i0:i0 + pt])
            x4 = x4_pool.tile([PT, FD], f16, tag="x4")
            nc.scalar.dma_start(out=x4[:pt, :], in_=x4d[i0:i0 + pt])
            xo = _XS_OFF[t]
            nc.scalar.dma_start(out=xs[1:rows, :], in_=xsd[xo:xo + 3 * nct])

            o = o_pool.tile([PT, FD], f16, tag="o")
            for c in range(FD // CC):
                sl = slice(CC * c, CC * (c + 1))
                ps = psum_pool.tile([PT, CC], f32, tag="ps")
                for m in range(CC // 512):
                    msl = slice(CC * c + 512 * m, CC * c + 512 * (m + 1))
                    nc.tensor.matmul(
                        ps[:pt, 512 * m:512 * (m + 1)],
                        lts[t][:],
                        xs[:rows, msl],
                        start=True, stop=True,
                    )
                b16 = b_pool.tile([PT, CC], f16, tag="b16")
                nc.scalar.copy(b16[:pt, :], ps[:pt, :])
                d = d_pool.tile([PT, CC], f16, tag="d")
                nc.vector.scalar_tensor_tensor(
                    d[:pt, :], x5[:pt, sl], w0ts[t][:pt, :], b16[:pt, :],
                    mult, add)
                nc.vector.scalar_tensor_tensor(
                    o[:pt, sl], x4[:pt, sl], w1ts[t][:pt, :], d[:pt, :],
                    mult, add)

            nc.sync.dma_start(out=outd[i0:i0 + pt], in_=o[:pt, :])

    nc.compile()
    return nc


def _get_program(w, b):
    key = (w.tobytes(), b.tobytes())
    if key not in _cache:
        _cache[key] = _build_program(w, b)
    return _cache[key]


def _pack_pairs(a):
    """[K, CH, FD] fp16 -> [NPAIR, FD] in chunk-major (chunk, k) pair order."""
    return np.ascontiguousarray(a.transpose(1, 0, 2).reshape(NPAIR, FD))


def run(inputs, trace=False, tmpdir=None):
    from concourse.bass_utils import run_bass_kernel_spmd

    w = np.asarray(inputs["weight"], dtype=np.float32)
    b = np.asarray(inputs["bias"], dtype=np.float32)
    nc = _get_program(w, b)

    s1f = np.asarray(inputs["side1"], dtype=np.float16).reshape(B, CH, FD)
    s2f = np.asarray(inputs["side2"], dtype=np.float16).reshape(B, CH, FD)
    s3f = np.asarray(inputs["side3"], dtype=np.float16).reshape(B, CH, FD)
    s4f = np.asarray(inputs["side4"], dtype=np.float16).reshape(B, K, CH, FD)
    s5f = np.asarray(inputs["side5"], dtype=np.float16).reshape(B, K, CH, FD)

    in_maps = []
    for c in range(N_CORES):
        xs_blocks = []
        for _, _, c0, nct in _TILES:
            xs_blocks += [s1f[c, c0:c0 + nct], s2f[c, c0:c0 + nct],
                          s3f[c, c0:c0 + nct]]
        in_maps.append({
            "x5": _pack_pairs(s5f[c]),
            "x4": _pack_pairs(s4f[c]),
            "xs": np.ascontiguousarray(np.concatenate(xs_blocks, axis=0)),
        })

    res = run_bass_kernel_spmd(nc, in_maps, list(range(N_CORES)),
                               trace=trace, tmpdir=tmpdir)
    outs = []
    for c in range(N_CORES):
        o = res.results[c]["out"]                      # [NPAIR, FD] fp16
        o = o.reshape(CH, K, FD).transpose(1, 0, 2)    # [K, CH, FD]
        outs.append(o.reshape(1, K, H, W).astype(np.float32))
    return np.concatenate(outs, axis=0), res


def kernel(**inputs):
    out, _ = run(inputs, trace=False)
    return out


# revision 7
# speedup vs baseline: 2.2758x; 1.0072x over previous
"""Trainium2 Bass kernel for nn_GroupedConvFuseSide4.

out[b,k] = w[k,0]*side5[b,k] + w[k,1]*side4[b,k]
         + w[k,2]*side1[b,0] + w[k,3]*side2[b,0] + w[k,4]*side3[b,0] + bias[k]

Sharding: pure data parallel over batch (B=8) across 8 NeuronCores.

Per-core scheme (fp16 staging, 128-partition packed pairs): the op is
memory-bound, so all large tensors are staged in DRAM as fp16 (host converts;
rel-err ~1e-3 vs the 2e-2 gate). The 262144 pixels of one image are split
into CH=32 chunks of FD=8192. The (chunk, k) pairs are enumerated
chunk-major into 608 rows; tiles take 128 consecutive rows (4 full tiles +
a 96-row tail), so every side5/side4/out DMA is one contiguous
[128, 16KB] = 2MB transfer with full 16-engine fanout.

Per tile: PE matmul (contraction = ones row + 3 singles x nct chunks, fp16,
zero-padded to a fixed 25 rows) computes base = w2*s1 + w3*s2 + w4*s3 + bias
into fp32 PSUM; the scalar engine (ACT) evacuates PSUM to fp16 SBUF; DVE
merges side5/side4 with tensor_scalar_mul (4x packed mode) + tensor_add
(2x) — scalar_tensor_tensor is avoided since it only runs in 1x mode.
The ones rows ride along in the singles DMA (no DVE memset). Weights/bias/
lhsT are baked into the program as inline const tensors.
"""

import numpy as np

B, K, H, W = 8, 19, 512, 512
NPIX = H * W               # 262144 pixels per channel image
FD = 8192                  # pixels per chunk
CH = NPIX // FD            # 32 chunks per image
NPAIR = CH * K             # 608 (chunk, k) pairs, chunk-major: i -> (i//K, i%K)
PT = 128                   # partitions per full tile
NT = NPAIR // PT           # 4 full tiles
TAIL = NPAIR - NT * PT     # 96-row tail tile
NTT = NT + 1               # 5 tiles total
MAXR = 25                  # fixed contraction rows (1 ones + 3*8 singles max)
CC = 2048                  # compute chunk (4 PSUM banks fp32)
N_CORES = 8

# per-tile geometry: (row offset, partitions, first chunk, n chunks)
_TILES = []
for _t in range(NTT):
    _i0 = _t * PT
    _pt = PT if _t < NT else TAIL
    _c0 = _i0 // K
    _c1 = (_i0 + _pt - 1) // K
    _TILES.append((_i0, _pt, _c0, _c1 - _c0 + 1))

_XS_OFF = []               # row offsets of each tile's block in xsall
_o = 0
for _, _, _, _nct in _TILES:
    _XS_OFF.append(_o)
    _o += 1 + 3 * _nct     # ones row + singles rows
XS_ROWS = _o               # 113

_cache = {}


def _build_program(w, b):
    import concourse.bacc as bacc
    import concourse.tile as tile
    import concourse.mybir as mybir
    from contextlib import ExitStack

    f16 = mybir.dt.float16
    f32 = mybir.dt.float32
    mult = mybir.AluOpType.mult
    add = mybir.AluOpType.add

    nc = bacc.Bacc(
        "TRN2", target_bir_lowering=False, debug=False,
        enable_asserts=False, num_devices=N_CORES,
    )

    x5d = nc.dram_tensor("x5", [NPAIR, FD], f16, kind="ExternalInput").ap()
    x4d = nc.dram_tensor("x4", [NPAIR, FD], f16, kind="ExternalInput").ap()
    xsd = nc.dram_tensor("xs", [XS_ROWS, FD], f16, kind="ExternalInput").ap()
    outd = nc.dram_tensor("out", [NPAIR, FD], f16, kind="ExternalOutput").ap()

    # ---- baked constants, consolidated into three inline tensors ----
    lhsT_all = np.zeros((MAXR, NTT * PT), dtype=np.float16)
    w0_all = np.zeros((PT, NTT), dtype=np.float32)
    w1_all = np.zeros((PT, NTT), dtype=np.float32)
    for t, (i0, pt, c0, nct) in enumerate(_TILES):
        for p in range(pt):
            i = i0 + p
            ch, k = i // K, i % K
            j = ch - c0
            col = t * PT + p
            lhsT_all[0, col] = b[k]
            lhsT_all[1 + 0 * nct + j, col] = w[k, 2]
            lhsT_all[1 + 1 * nct + j, col] = w[k, 3]
            lhsT_all[1 + 2 * nct + j, col] = w[k, 4]
            w0_all[p, t] = w[k, 0]
            w1_all[p, t] = w[k, 1]
    lhsT_d = nc.inline_tensor(lhsT_all, name="lhsT").ap()
    w0_d = nc.inline_tensor(w0_all, name="w0v").ap()
    w1_d = nc.inline_tensor(w1_all, name="w1v").ap()

    with tile.TileContext(nc) as tc, ExitStack() as ctx:
        consts = ctx.enter_context(tc.tile_pool(name="consts", bufs=1))
        xs_pool = ctx.enter_context(tc.tile_pool(name="xs", bufs=1))
        x5_pool = ctx.enter_context(tc.tile_pool(name="x5", bufs=3))
        x4_pool = ctx.enter_context(tc.tile_pool(name="x4", bufs=3))
        b_pool = ctx.enter_context(tc.tile_pool(name="bb", bufs=2))
        t5_pool = ctx.enter_context(tc.tile_pool(name="t5", bufs=2))
        t4_pool = ctx.enter_context(tc.tile_pool(name="t4", bufs=2))
        u_pool = ctx.enter_context(tc.tile_pool(name="u", bufs=2))
        o_pool = ctx.enter_context(tc.tile_pool(name="o", bufs=2))
        psum_pool = ctx.enter_context(tc.tile_pool(name="ps", bufs=2, space="PSUM"))

        lt = consts.tile([MAXR, NTT * PT], f16, tag="lt")
        w0t = consts.tile([PT, NTT], f32, tag="w0t")
        w1t = consts.tile([PT, NTT], f32, tag="w1t")
        nc.sync.dma_start(out=lt[:], in_=lhsT_d)
        nc.sync.dma_start(out=w0t[:], in_=w0_d)
        nc.sync.dma_start(out=w1t[:], in_=w1_d)

        # singles tiles (ring of 2); row 0 = ones (comes in via the DMA)
        xs_tiles = [xs_pool.tile([MAXR, FD], f16, tag=f"xs{i}", name=f"xs{i}")
                    for i in range(2)]

        for t in range(NTT):
            i0, pt, c0, nct = _TILES[t]
            rows = 1 + 3 * nct
            xs = xs_tiles[t % 2]

            x5 = x5_pool.tile([PT, FD], f16, tag="x5")
            nc.sync.dma_start(out=x5[:pt, :], in_=x5d[i0:i0 + pt])
            x4 = x4_pool.tile([PT, FD], f16, tag="x4")
            nc.scalar.dma_start(out=x4[:pt, :], in_=x4d[i0:i0 + pt])
            xo = _XS_OFF[t]
            nc.scalar.dma_start(out=xs[0:rows, :], in_=xsd[xo:xo + rows])

            o = o_pool.tile([PT, FD], f16, tag="o")
            for c in range(FD // CC):
                sl = slice(CC * c, CC * (c + 1))
                ps = psum_pool.tile([PT, CC], f32, tag="ps")
                for m in range(CC // 512):
                    msl = slice(CC * c + 512 * m, CC * c + 512 * (m + 1))
                    nc.tensor.matmul(
                        ps[:pt, 512 * m:512 * (m + 1)],
                        lt[:rows, t * PT:t * PT + pt],
                        xs[:rows, msl],
                        start=True, stop=True,
                    )
                b16 = b_pool.tile([PT, CC], f16, tag="b16")
                nc.scalar.copy(b16[:pt, :], ps[:pt, :])
                t5 = t5_pool.tile([PT, CC], f16, tag="t5")
                nc.vector.tensor_scalar_mul(
                    t5[:pt, :], x5[:pt, sl], w0t[:pt, t:t + 1])
                t4 = t4_pool.tile([PT, CC], f16, tag="t4")
                nc.vector.tensor_scalar_mul(
                    t4[:pt, :], x4[:pt, sl], w1t[:pt, t:t + 1])
                u = u_pool.tile([PT, CC], f16, tag="u")
                nc.vector.tensor_add(u[:pt, :], t5[:pt, :], t4[:pt, :])
                nc.vector.tensor_add(o[:pt, sl], u[:pt, :], b16[:pt, :])

            nc.sync.dma_start(out=outd[i0:i0 + pt], in_=o[:pt, :])

    nc.compile()
    return nc


def _get_program(w, b):
    key = (w.tobytes(), b.tobytes())
    if key not in _cache:
        _cache[key] = _build_program(w, b)
    return _cache[key]


def _pack_pairs(a):
    """[K, CH, FD] fp16 -> [NPAIR, FD] in chunk-major (chunk, k) pair order."""
    return np.ascontiguousarray(a.transpose(1, 0, 2).reshape(NPAIR, FD))


def run(inputs, trace=False, tmpdir=None):
    from concourse.bass_utils import run_bass_kernel_spmd

    w = np.asarray(inputs["weight"], dtype=np.float32)
    b = np.asarray(inputs["bias"], dtype=np.float32)
    nc = _get_program(w, b)

    s1f = np.asarray(inputs["side1"], dtype=np.float16).reshape(B, CH, FD)
    s2f = np.asarray(inputs["side2"], dtype=np.float16).reshape(B, CH, FD)
    s3f = np.asarray(inputs["side3"], dtype=np.float16).reshape(B, CH, FD)
    s4f = np.asarray(inputs["side4"], dtype=np.float16).reshape(B, K, CH, FD)
    s5f = np.asarray(inputs["side5"], dtype=np.float16).reshape(B, K, CH, FD)
    ones = np.ones((1, FD), dtype=np.float16)

    in_maps = []
    for c in range(N_CORES):
        xs_blocks = []
        for _, _, c0, nct in _TILES:
            xs_blocks += [ones, s1f[c, c0:c0 + nct], s2f[c, c0:c0 + nct],
                          s3f[c, c0:c0 + nct]]
        in_maps.append({
            "x5": _pack_pairs(s5f[c]),
            "x4": _pack_pairs(s4f[c]),
            "xs": np.ascontiguousarray(np.concatenate(xs_blocks, axis=0)),
        })

    res = run_bass_kernel_spmd(nc, in_maps, list(range(N_CORES)),
                               trace=trace, tmpdir=tmpdir)
    outs = []
    for c in range(N_CORES):
        o = res.results[c]["out"]                      # [NPAIR, FD] fp16
        o = o.reshape(CH, K, FD).transpose(1, 0, 2)    # [K, CH, FD]
        outs.append(o.reshape(1, K, H, W).astype(np.float32))
    return np.concatenate(outs, axis=0), res


def kernel(**inputs):
    out, _ = run(inputs, trace=False)
    return out


# revision 9
# speedup vs baseline: 2.2814x; 1.0025x over previous
"""Trainium2 Bass kernel for nn_GroupedConvFuseSide4.

out[b,k] = w[k,0]*side5[b,k] + w[k,1]*side4[b,k]
         + w[k,2]*side1[b,0] + w[k,3]*side2[b,0] + w[k,4]*side3[b,0] + bias[k]

Sharding: pure data parallel over batch (B=8) across 8 NeuronCores.

Per-core scheme (fp16 staging, 128-partition packed pairs): the op is
memory-bound, so all large tensors are staged in DRAM as fp16 (host converts;
rel-err ~1e-3 vs the 2e-2 gate). The 262144 pixels of one image are split
into CH=32 chunks of FD=8192. The (chunk, k) pairs are enumerated
chunk-major into 608 rows; tiles take 128 consecutive rows (4 full tiles +
a 96-row tail), so every side5/side4/out DMA is one contiguous
[128, 16KB] = 2MB transfer with full 16-engine fanout.

Per tile: PE matmul (contraction = ones row + 3 singles x nct chunks, fp16,
zero-padded to a fixed 25 rows) computes base = w2*s1 + w3*s2 + w4*s3 + bias
into fp32 PSUM; the scalar engine (ACT) evacuates PSUM to fp16 SBUF; DVE
merges side5/side4 with tensor_scalar_mul (4x packed mode) + tensor_add
(2x) — scalar_tensor_tensor is avoided since it only runs in 1x mode.
The ones rows ride along in the singles DMA (no DVE memset). Weights/bias/
lhsT are baked into the program as inline const tensors.
"""

import numpy as np

B, K, H, W = 8, 19, 512, 512
NPIX = H * W               # 262144 pixels per channel image
FD = 8192                  # pixels per chunk
CH = NPIX // FD            # 32 chunks per image
NPAIR = CH * K             # 608 (chunk, k) pairs, chunk-major: i -> (i//K, i%K)
PT = 128                   # partitions per full tile
NT = NPAIR // PT           # 4 full tiles
TAIL = NPAIR - NT * PT     # 96-row tail tile
NTT = NT + 1               # 5 tiles total
MAXR = 25                  # fixed contraction rows (1 ones + 3*8 singles max)
CC = 2048                  # compute chunk (4 PSUM banks fp32)
N_CORES = 8

# per-tile geometry: (row offset, partitions, first chunk, n chunks)
_TILES = []
for _t in range(NTT):
    _i0 = _t * PT
    _pt = PT if _t < NT else TAIL
    _c0 = _i0 // K
    _c1 = (_i0 + _pt - 1) // K
    _TILES.append((_i0, _pt, _c0, _c1 - _c0 + 1))

_XS_OFF = []               # row offsets of each tile's block in xsall
_o = 0
for _, _, _, _nct in _TILES:
    _XS_OFF.append(_o)
    _o += 1 + 3 * _nct     # ones row + singles rows
XS_ROWS = _o               # 113

_cache = {}


def _build_program(w, b):
    import concourse.bacc as bacc
    import concourse.tile as tile
    import concourse.mybir as mybir
    from contextlib import ExitStack

    f16 = mybir.dt.float16
    f32 = mybir.dt.float32
    mult = mybir.AluOpType.mult
    add = mybir.AluOpType.add

    nc = bacc.Bacc(
        "TRN2", target_bir_lowering=False, debug=False,
        enable_asserts=False, num_devices=N_CORES,
    )

    x5d = nc.dram_tensor("x5", [NPAIR, FD], f16, kind="ExternalInput").ap()
    x4d = nc.dram_tensor("x4", [NPAIR, FD], f16, kind="ExternalInput").ap()
    xsd = nc.dram_tensor("xs", [XS_ROWS, FD], f16, kind="ExternalInput").ap()
    outd = nc.dram_tensor("out", [NPAIR, FD], f16, kind="ExternalOutput").ap()

    # ---- baked constants, consolidated into three inline tensors ----
    lhsT_all = np.zeros((MAXR, NTT * PT), dtype=np.float16)
    w0_all = np.zeros((PT, NTT), dtype=np.float32)
    w1_all = np.zeros((PT, NTT), dtype=np.float32)
    for t, (i0, pt, c0, nct) in enumerate(_TILES):
        for p in range(pt):
            i = i0 + p
            ch, k = i // K, i % K
            j = ch - c0
            col = t * PT + p
            lhsT_all[0, col] = b[k]
            lhsT_all[1 + 0 * nct + j, col] = w[k, 2]
            lhsT_all[1 + 1 * nct + j, col] = w[k, 3]
            lhsT_all[1 + 2 * nct + j, col] = w[k, 4]
            w0_all[p, t] = w[k, 0]
            w1_all[p, t] = w[k, 1]
    lhsT_d = nc.inline_tensor(lhsT_all, name="lhsT").ap()
    w0_d = nc.inline_tensor(w0_all, name="w0v").ap()
    w1_d = nc.inline_tensor(w1_all, name="w1v").ap()

    with tile.TileContext(nc) as tc, ExitStack() as ctx:
        consts = ctx.enter_context(tc.tile_pool(name="consts", bufs=1))
        xs_pool = ctx.enter_context(tc.tile_pool(name="xs", bufs=1))
        x5_pool = ctx.enter_context(tc.tile_pool(name="x5", bufs=3))
        x4_pool = ctx.enter_context(tc.tile_pool(name="x4", bufs=3))
        b_pool = ctx.enter_context(tc.tile_pool(name="bb", bufs=2))
        o_pool = ctx.enter_context(tc.tile_pool(name="o", bufs=2))
        psum_pool = ctx.enter_context(tc.tile_pool(name="ps", bufs=2, space="PSUM"))

        lt = consts.tile([MAXR, NTT * PT], f16, tag="lt")
        w0t = consts.tile([PT, NTT], f32, tag="w0t")
        w1t = consts.tile([PT, NTT], f32, tag="w1t")
        nc.sync.dma_start(out=lt[:], in_=lhsT_d)
        nc.sync.dma_start(out=w0t[:], in_=w0_d)
        nc.sync.dma_start(out=w1t[:], in_=w1_d)

        # singles tiles (ring of 2); row 0 = ones (comes in via the DMA)
        xs_tiles = [xs_pool.tile([MAXR, FD], f16, tag=f"xs{i}", name=f"xs{i}")
                    for i in range(2)]

        for t in range(NTT):
            i0, pt, c0, nct = _TILES[t]
            rows = 1 + 3 * nct
            xs = xs_tiles[t % 2]

            x5 = x5_pool.tile([PT, FD], f16, tag="x5")
            nc.sync.dma_start(out=x5[:pt, :], in_=x5d[i0:i0 + pt])
            x4 = x4_pool.tile([PT, FD], f16, tag="x4")
            nc.scalar.dma_start(out=x4[:pt, :], in_=x4d[i0:i0 + pt])
            xo = _XS_OFF[t]
            nc.scalar.dma_start(out=xs[0:rows, :], in_=xsd[xo:xo + rows])

            # full-width in-place DVE ops (amortize the per-instruction bubble)
            nc.vector.tensor_scalar_mul(
                x5[:pt, :], x5[:pt, :], w0t[:pt, t:t + 1])
            nc.vector.tensor_scalar_mul(
                x4[:pt, :], x4[:pt, :], w1t[:pt, t:t + 1])
            nc.vector.tensor_add(x4[:pt, :], x5[:pt, :], x4[:pt, :])

            b16 = b_pool.tile([PT, FD], f16, tag="b16")
            for c in range(FD // CC):
                sl = slice(CC * c, CC * (c + 1))
                ps = psum_pool.tile([PT, CC], f32, tag="ps")
                for m in range(CC // 512):
                    msl = slice(CC * c + 512 * m, CC * c + 512 * (m + 1))
                    nc.tensor.matmul(
                        ps[:pt, 512 * m:512 * (m + 1)],
                        lt[:rows, t * PT:t * PT + pt],
                        xs[:rows, msl],
                        start=True, stop=True,
                    )
                nc.scalar.copy(b16[:pt, sl], ps[:pt, :])

            o = o_pool.tile([PT, FD], f16, tag="o")
            nc.vector.tensor_add(o[:pt, :], x4[:pt, :], b16[:pt, :])

            nc.sync.dma_start(out=outd[i0:i0 + pt], in_=o[:pt, :])

    nc.compile()
    return nc


def _get_program(w, b):
    key = (w.tobytes(), b.tobytes())
    if key not in _cache:
        _cache[key] = _build_program(w, b)
    return _cache[key]


def _pack_pairs(a):
    """[K, CH, FD] fp16 -> [NPAIR, FD] in chunk-major (chunk, k) pair order."""
    return np.ascontiguousarray(a.transpose(1, 0, 2).reshape(NPAIR, FD))


def run(inputs, trace=False, tmpdir=None):
    from concourse.bass_utils import run_bass_kernel_spmd

    w = np.asarray(inputs["weight"], dtype=np.float32)
    b = np.asarray(inputs["bias"], dtype=np.float32)
    nc = _get_program(w, b)

    s1f = np.asarray(inputs["side1"], dtype=np.float16).reshape(B, CH, FD)
    s2f = np.asarray(inputs["side2"], dtype=np.float16).reshape(B, CH, FD)
    s3f = np.asarray(inputs["side3"], dtype=np.float16).reshape(B, CH, FD)
    s4f = np.asarray(inputs["side4"], dtype=np.float16).reshape(B, K, CH, FD)
    s5f = np.asarray(inputs["side5"], dtype=np.float16).reshape(B, K, CH, FD)
    ones = np.ones((1, FD), dtype=np.float16)

    in_maps = []
    for c in range(N_CORES):
        xs_blocks = []
        for _, _, c0, nct in _TILES:
            xs_blocks += [ones, s1f[c, c0:c0 + nct], s2f[c, c0:c0 + nct],
                          s3f[c, c0:c0 + nct]]
        in_maps.append({
            "x5": _pack_pairs(s5f[c]),
            "x4": _pack_pairs(s4f[c]),
            "xs": np.ascontiguousarray(np.concatenate(xs_blocks, axis=0)),
        })

    res = run_bass_kernel_spmd(nc, in_maps, list(range(N_CORES)),
                               trace=trace, tmpdir=tmpdir)
    outs = []
    for c in range(N_CORES):
        o = res.results[c]["out"]                      # [NPAIR, FD] fp16
        o = o.reshape(CH, K, FD).transpose(1, 0, 2)    # [K, CH, FD]
        outs.append(o.reshape(1, K, H, W).astype(np.float32))
    return np.concatenate(outs, axis=0), res


def kernel(**inputs):
    out, _ = run(inputs, trace=False)
    return out


# revision 13
# speedup vs baseline: 2.5621x; 1.1230x over previous
"""Trainium2 Bass kernel for nn_GroupedConvFuseSide4.

out[b,k] = w[k,0]*side5[b,k] + w[k,1]*side4[b,k]
         + w[k,2]*side1[b,0] + w[k,3]*side2[b,0] + w[k,4]*side3[b,0] + bias[k]

Sharding: pure data parallel over batch (B=8) across 8 NeuronCores.

Per-core scheme (fp16 staging, 128-partition packed pairs): the op is
memory-bound, so all large tensors are staged in DRAM as fp16 (host converts;
rel-err ~1e-3 vs the 2e-2 gate). The 262144 pixels of one image are split
into CH=32 chunks of FD=8192. The (chunk, k) pairs are enumerated
chunk-major into 608 rows; tiles take 128 consecutive rows (4 full tiles +
a 96-row tail), so every side5/side4/out DMA is one contiguous
[128, 16KB] = 2MB transfer with full 16-engine fanout.

Per tile: PE matmul (contraction = ones row + 3 singles x nct chunks, fp16,
zero-padded to a fixed 25 rows) computes base = w2*s1 + w3*s2 + w4*s3 + bias
into fp32 PSUM; the scalar engine (ACT) evacuates PSUM to fp16 SBUF; DVE
merges side5/side4 with tensor_scalar_mul (4x packed mode) + tensor_add
(2x) — scalar_tensor_tensor is avoided since it only runs in 1x mode.
The ones rows ride along in the singles DMA (no DVE memset). Weights/bias/
lhsT are baked into the program as inline const tensors.
"""

import numpy as np

B, K, H, W = 8, 19, 512, 512
NPIX = H * W               # 262144 pixels per channel image
FD = 4096                  # pixels per chunk
CH = NPIX // FD            # 32 chunks per image
NPAIR = CH * K             # 608 (chunk, k) pairs, chunk-major: i -> (i//K, i%K)
PT = 128                   # partitions per full tile
NT = NPAIR // PT           # 4 full tiles
TAIL = NPAIR - NT * PT     # 96-row tail tile
NTT = NT + 1               # 5 tiles total
MAXR = 25                  # fixed contraction rows (1 ones + 3*8 singles max)
CC = 2048                  # compute chunk (4 PSUM banks fp32)
N_CORES = 8

# per-tile geometry: (row offset, partitions, first chunk, n chunks)
_TILES = []
for _t in range(NTT):
    _i0 = _t * PT
    _pt = PT if _t < NT else TAIL
    _c0 = _i0 // K
    _c1 = (_i0 + _pt - 1) // K
    _TILES.append((_i0, _pt, _c0, _c1 - _c0 + 1))

_XS_OFF = []               # row offsets of each tile's block in xsall
_o = 0
for _, _, _, _nct in _TILES:
    _XS_OFF.append(_o)
    _o += 1 + 3 * _nct     # ones row + singles rows
XS_ROWS = _o               # 113

_cache = {}


def _build_program(w, b):
    import concourse.bacc as bacc
    import concourse.tile as tile
    import concourse.mybir as mybir
    from contextlib import ExitStack

    f16 = mybir.dt.float16
    f32 = mybir.dt.float32
    mult = mybir.AluOpType.mult
    add = mybir.AluOpType.add

    nc = bacc.Bacc(
        "TRN2", target_bir_lowering=False, debug=False,
        enable_asserts=False, num_devices=N_CORES,
    )

    x5d = nc.dram_tensor("x5", [NPAIR, FD], f16, kind="ExternalInput").ap()
    x4d = nc.dram_tensor("x4", [NPAIR, FD], f16, kind="ExternalInput").ap()
    xsd = nc.dram_tensor("xs", [XS_ROWS, FD], f16, kind="ExternalInput").ap()
    outd = nc.dram_tensor("out", [NPAIR, FD], f16, kind="ExternalOutput").ap()

    # ---- baked constants, consolidated into three inline tensors ----
    lhsT_all = np.zeros((MAXR, NTT * PT), dtype=np.float16)
    w0_all = np.zeros((PT, NTT), dtype=np.float32)
    w1_all = np.zeros((PT, NTT), dtype=np.float32)
    for t, (i0, pt, c0, nct) in enumerate(_TILES):
        for p in range(pt):
            i = i0 + p
            ch, k = i // K, i % K
            j = ch - c0
            col = t * PT + p
            lhsT_all[0, col] = b[k]
            lhsT_all[1 + 0 * nct + j, col] = w[k, 2]
            lhsT_all[1 + 1 * nct + j, col] = w[k, 3]
            lhsT_all[1 + 2 * nct + j, col] = w[k, 4]
            w0_all[p, t] = w[k, 0]
            w1_all[p, t] = w[k, 1]
    lhsT_d = nc.inline_tensor(lhsT_all, name="lhsT").ap()
    w0_d = nc.inline_tensor(w0_all, name="w0v").ap()
    w1_d = nc.inline_tensor(w1_all, name="w1v").ap()

    with tile.TileContext(nc) as tc, ExitStack() as ctx:
        consts = ctx.enter_context(tc.tile_pool(name="consts", bufs=1))
        xs_pool = ctx.enter_context(tc.tile_pool(name="xs", bufs=1))
        x5_pool = ctx.enter_context(tc.tile_pool(name="x5", bufs=4))
        x4_pool = ctx.enter_context(tc.tile_pool(name="x4", bufs=4))
        b_pool = ctx.enter_context(tc.tile_pool(name="bb", bufs=3))
        o_pool = ctx.enter_context(tc.tile_pool(name="o", bufs=3))
        psum_pool = ctx.enter_context(tc.tile_pool(name="ps", bufs=2, space="PSUM"))

        lt = consts.tile([MAXR, NTT * PT], f16, tag="lt")
        w0t = consts.tile([PT, NTT], f32, tag="w0t")
        w1t = consts.tile([PT, NTT], f32, tag="w1t")
        nc.sync.dma_start(out=lt[:], in_=lhsT_d)
        nc.sync.dma_start(out=w0t[:], in_=w0_d)
        nc.sync.dma_start(out=w1t[:], in_=w1_d)

        # singles tiles (ring of 2); row 0 = ones (comes in via the DMA)
        xs_tiles = [xs_pool.tile([MAXR, FD], f16, tag=f"xs{i}", name=f"xs{i}")
                    for i in range(3)]

        for t in range(NTT):
            i0, pt, c0, nct = _TILES[t]
            rows = 1 + 3 * nct
            xs = xs_tiles[t % 3]

            x5 = x5_pool.tile([PT, FD], f16, tag="x5")
            nc.sync.dma_start(out=x5[:pt, :], in_=x5d[i0:i0 + pt])
            x4 = x4_pool.tile([PT, FD], f16, tag="x4")
            nc.scalar.dma_start(out=x4[:pt, :], in_=x4d[i0:i0 + pt])
            xo = _XS_OFF[t]
            nc.scalar.dma_start(out=xs[0:rows, :], in_=xsd[xo:xo + rows])

            # full-width in-place DVE ops (amortize the per-instruction bubble)
            nc.vector.tensor_scalar_mul(
                x5[:pt, :], x5[:pt, :], w0t[:pt, t:t + 1])
            nc.vector.tensor_scalar_mul(
                x4[:pt, :], x4[:pt, :], w1t[:pt, t:t + 1])
            nc.vector.tensor_add(x4[:pt, :], x5[:pt, :], x4[:pt, :])

            b16 = b_pool.tile([PT, FD], f16, tag="b16")
            for c in range(FD // CC):
                sl = slice(CC * c, CC * (c + 1))
                ps = psum_pool.tile([PT, CC], f32, tag="ps")
                for m in range(CC // 512):
                    msl = slice(CC * c + 512 * m, CC * c + 512 * (m + 1))
                    nc.tensor.matmul(
                        ps[:pt, 512 * m:512 * (m + 1)],
                        lt[:rows, t * PT:t * PT + pt],
                        xs[:rows, msl],
                        start=True, stop=True,
                    )
                nc.scalar.copy(b16[:pt, sl], ps[:pt, :])

            o = o_pool.tile([PT, FD], f16, tag="o")
            nc.vector.tensor_add(o[:pt, :], x4[:pt, :], b16[:pt, :])

            nc.sync.dma_start(out=outd[i0:i0 + pt], in_=o[:pt, :])

    nc.compile()
    return nc


def _get_program(w, b):
    key = (w.tobytes(), b.tobytes())
    if key not in _cache:
        _cache[key] = _build_program(w, b)
    return _cache[key]


def _pack_pairs(a):
    """[K, CH, FD] fp16 -> [NPAIR, FD] in chunk-major (chunk, k) pair order."""
    return np.ascontiguousarray(a.transpose(1, 0, 2).reshape(NPAIR, FD))


def run(inputs, trace=False, tmpdir=None):
    from concourse.bass_utils import run_bass_kernel_spmd

    w = np.asarray(inputs["weight"], dtype=np.float32)
    b = np.asarray(inputs["bias"], dtype=np.float32)
    nc = _get_program(w, b)

    s1f = np.asarray(inputs["side1"], dtype=np.float16).reshape(B, CH, FD)
    s2f = np.asarray(inputs["side2"], dtype=np.float16).reshape(B, CH, FD)
    s3f = np.asarray(inputs["side3"], dtype=np.float16).reshape(B, CH, FD)
    s4f = np.asarray(inputs["side4"], dtype=np.float16).reshape(B, K, CH, FD)
    s5f = np.asarray(inputs["side5"], dtype=np.float16).reshape(B, K, CH, FD)
    ones = np.ones((1, FD), dtype=np.float16)

    in_maps = []
    for c in range(N_CORES):
        xs_blocks = []
        for _, _, c0, nct in _TILES:
            xs_blocks += [ones, s1f[c, c0:c0 + nct], s2f[c, c0:c0 + nct],
                          s3f[c, c0:c0 + nct]]
        in_maps.append({
            "x5": _pack_pairs(s5f[c]),
            "x4": _pack_pairs(s4f[c]),
            "xs": np.ascontiguousarray(np.concatenate(xs_blocks, axis=0)),
        })

    res = run_bass_kernel_spmd(nc, in_maps, list(range(N_CORES)),
                               trace=trace, tmpdir=tmpdir)
    outs = []
    for c in range(N_CORES):
        o = res.results[c]["out"]                      # [NPAIR, FD] fp16
        o = o.reshape(CH, K, FD).transpose(1, 0, 2)    # [K, CH, FD]
        outs.append(o.reshape(1, K, H, W).astype(np.float32))
    return np.concatenate(outs, axis=0), res


def kernel(**inputs):
    out, _ = run(inputs, trace=False)
    return out


# revision 15
# speedup vs baseline: 2.6027x; 1.0158x over previous
"""Trainium2 Bass kernel for nn_GroupedConvFuseSide4.

out[b,k] = w[k,0]*side5[b,k] + w[k,1]*side4[b,k]
         + w[k,2]*side1[b,0] + w[k,3]*side2[b,0] + w[k,4]*side3[b,0] + bias[k]

Sharding: pure data parallel over batch (B=8) across 8 NeuronCores.

Per-core scheme (fp16 staging, 128-partition packed pairs): the op is
memory-bound, so all large tensors are staged in DRAM as fp16 (host converts;
rel-err ~1e-3 vs the 2e-2 gate). The 262144 pixels of one image are split
into CH=32 chunks of FD=8192. The (chunk, k) pairs are enumerated
chunk-major into 608 rows; tiles take 128 consecutive rows (4 full tiles +
a 96-row tail), so every side5/side4/out DMA is one contiguous
[128, 16KB] = 2MB transfer with full 16-engine fanout.

Per tile: PE matmul (contraction = ones row + 3 singles x nct chunks, fp16,
zero-padded to a fixed 25 rows) computes base = w2*s1 + w3*s2 + w4*s3 + bias
into fp32 PSUM; the scalar engine (ACT) evacuates PSUM to fp16 SBUF; DVE
merges side5/side4 with tensor_scalar_mul (4x packed mode) + tensor_add
(2x) — scalar_tensor_tensor is avoided since it only runs in 1x mode.
The ones rows ride along in the singles DMA (no DVE memset). Weights/bias/
lhsT are baked into the program as inline const tensors.
"""

import numpy as np

B, K, H, W = 8, 19, 512, 512
NPIX = H * W               # 262144 pixels per channel image
FD = 4096                  # pixels per chunk
CH = NPIX // FD            # 32 chunks per image
NPAIR = CH * K             # 608 (chunk, k) pairs, chunk-major: i -> (i//K, i%K)
PT = 128                   # partitions per full tile
NT = NPAIR // PT           # 4 full tiles
TAIL = NPAIR - NT * PT     # 96-row tail tile
NTT = NT + 1               # 5 tiles total
MAXR = 25                  # fixed contraction rows (1 ones + 3*8 singles max)
CC = 2048                  # compute chunk (4 PSUM banks fp32)
N_CORES = 8

# per-tile geometry: (row offset, partitions, first chunk, n chunks)
_TILES = []
for _t in range(NTT):
    _i0 = _t * PT
    _pt = PT if _t < NT else TAIL
    _c0 = _i0 // K
    _c1 = (_i0 + _pt - 1) // K
    _TILES.append((_i0, _pt, _c0, _c1 - _c0 + 1))

_XS_OFF = []               # row offsets of each tile's block in xsall
_o = 0
for _, _, _, _nct in _TILES:
    _XS_OFF.append(_o)
    _o += 1 + 3 * _nct     # ones row + singles rows
XS_ROWS = _o               # 113

_cache = {}


def _build_program(w, b):
    import concourse.bacc as bacc
    import concourse.tile as tile
    import concourse.mybir as mybir
    from contextlib import ExitStack

    f16 = mybir.dt.float16
    f32 = mybir.dt.float32
    mult = mybir.AluOpType.mult
    add = mybir.AluOpType.add

    nc = bacc.Bacc(
        "TRN2", target_bir_lowering=False, debug=False,
        enable_asserts=False, num_devices=N_CORES,
    )

    x5d = nc.dram_tensor("x5", [NPAIR, FD], f16, kind="ExternalInput").ap()
    x4d = nc.dram_tensor("x4", [NPAIR, FD], f16, kind="ExternalInput").ap()
    xsd = nc.dram_tensor("xs", [XS_ROWS, FD], f16, kind="ExternalInput").ap()
    outd = nc.dram_tensor("out", [NPAIR, FD], f16, kind="ExternalOutput").ap()

    # ---- baked constants, consolidated into three inline tensors ----
    lhsT_all = np.zeros((MAXR, NTT * PT), dtype=np.float16)
    w0_all = np.zeros((PT, NTT), dtype=np.float32)
    w1_all = np.zeros((PT, NTT), dtype=np.float32)
    for t, (i0, pt, c0, nct) in enumerate(_TILES):
        for p in range(pt):
            i = i0 + p
            ch, k = i // K, i % K
            j = ch - c0
            col = t * PT + p
            lhsT_all[0, col] = b[k]
            lhsT_all[1 + 0 * nct + j, col] = w[k, 2]
            lhsT_all[1 + 1 * nct + j, col] = w[k, 3]
            lhsT_all[1 + 2 * nct + j, col] = w[k, 4]
            w0_all[p, t] = w[k, 0]
            w1_all[p, t] = w[k, 1]
    lhsT_d = nc.inline_tensor(lhsT_all, name="lhsT").ap()
    w0_d = nc.inline_tensor(w0_all, name="w0v").ap()
    w1_d = nc.inline_tensor(w1_all, name="w1v").ap()

    with tile.TileContext(nc) as tc, ExitStack() as ctx:
        consts = ctx.enter_context(tc.tile_pool(name="consts", bufs=1))
        xs_pool = ctx.enter_context(tc.tile_pool(name="xs", bufs=1))
        x5_pool = ctx.enter_context(tc.tile_pool(name="x5", bufs=4))
        x4_pool = ctx.enter_context(tc.tile_pool(name="x4", bufs=4))
        b_pool = ctx.enter_context(tc.tile_pool(name="bb", bufs=3))
        o_pool = ctx.enter_context(tc.tile_pool(name="o", bufs=3))
        psum_pool = ctx.enter_context(tc.tile_pool(name="ps", bufs=2, space="PSUM"))

        lt = consts.tile([MAXR, NTT * PT], f16, tag="lt")
        w0t = consts.tile([PT, NTT], f32, tag="w0t")
        w1t = consts.tile([PT, NTT], f32, tag="w1t")
        nc.sync.dma_start(out=lt[:], in_=lhsT_d)
        nc.sync.dma_start(out=w0t[:], in_=w0_d)
        nc.sync.dma_start(out=w1t[:], in_=w1_d)

        # singles tiles (ring of 2); row 0 = ones (comes in via the DMA)
        xs_tiles = [xs_pool.tile([MAXR, FD], f16, tag=f"xs{i}", name=f"xs{i}")
                    for i in range(3)]

        for t in range(NTT):
            i0, pt, c0, nct = _TILES[t]
            rows = 1 + 3 * nct
            xs = xs_tiles[t % 3]

            xo = _XS_OFF[t]
            nc.sync.dma_start(out=xs[0:rows, :], in_=xsd[xo:xo + rows])
            x5 = x5_pool.tile([PT, FD], f16, tag="x5")
            nc.sync.dma_start(out=x5[:pt, :], in_=x5d[i0:i0 + pt])
            x4 = x4_pool.tile([PT, FD], f16, tag="x4")
            nc.sync.dma_start(out=x4[:pt, :], in_=x4d[i0:i0 + pt])

            # full-width in-place DVE ops (amortize the per-instruction bubble)
            nc.vector.tensor_scalar_mul(
                x5[:pt, :], x5[:pt, :], w0t[:pt, t:t + 1])
            nc.vector.tensor_scalar_mul(
                x4[:pt, :], x4[:pt, :], w1t[:pt, t:t + 1])
            nc.vector.tensor_add(x4[:pt, :], x5[:pt, :], x4[:pt, :])

            b16 = b_pool.tile([PT, FD], f16, tag="b16")
            for c in range(FD // CC):
                sl = slice(CC * c, CC * (c + 1))
                ps = psum_pool.tile([PT, CC], f32, tag="ps")
                for m in range(CC // 512):
                    msl = slice(CC * c + 512 * m, CC * c + 512 * (m + 1))
                    nc.tensor.matmul(
                        ps[:pt, 512 * m:512 * (m + 1)],
                        lt[:rows, t * PT:t * PT + pt],
                        xs[:rows, msl],
                        start=True, stop=True,
                    )
                nc.scalar.copy(b16[:pt, sl], ps[:pt, :])

            o = o_pool.tile([PT, FD], f16, tag="o")
            nc.vector.tensor_add(o[:pt, :], x4[:pt, :], b16[:pt, :])

            nc.gpsimd.dma_start(out=outd[i0:i0 + pt], in_=o[:pt, :])

    nc.compile()
    return nc


def _get_program(w, b):
    key = (w.tobytes(), b.tobytes())
    if key not in _cache:
        _cache[key] = _build_program(w, b)
    return _cache[key]


def _pack_pairs(a):
    """[K, CH, FD] fp16 -> [NPAIR, FD] in chunk-major (chunk, k) pair order."""
    return np.ascontiguousarray(a.transpose(1, 0, 2).reshape(NPAIR, FD))


def run(inputs, trace=False, tmpdir=None):
    from concourse.bass_utils import run_bass_kernel_spmd

    w = np.asarray(inputs["weight"], dtype=np.float32)
    b = np.asarray(inputs["bias"], dtype=np.float32)
    nc = _get_program(w, b)

    s1f = np.asarray(inputs["side1"], dtype=np.float16).reshape(B, CH, FD)
    s2f = np.asarray(inputs["side2"], dtype=np.float16).reshape(B, CH, FD)
    s3f = np.asarray(inputs["side3"], dtype=np.float16).reshape(B, CH, FD)
    s4f = np.asarray(inputs["side4"], dtype=np.float16).reshape(B, K, CH, FD)
    s5f = np.asarray(inputs["side5"], dtype=np.float16).reshape(B, K, CH, FD)
    ones = np.ones((1, FD), dtype=np.float16)

    in_maps = []
    for c in range(N_CORES):
        xs_blocks = []
        for _, _, c0, nct in _TILES:
            xs_blocks += [ones, s1f[c, c0:c0 + nct], s2f[c, c0:c0 + nct],
                          s3f[c, c0:c0 + nct]]
        in_maps.append({
            "x5": _pack_pairs(s5f[c]),
            "x4": _pack_pairs(s4f[c]),
            "xs": np.ascontiguousarray(np.concatenate(xs_blocks, axis=0)),
        })

    res = run_bass_kernel_spmd(nc, in_maps, list(range(N_CORES)),
                               trace=trace, tmpdir=tmpdir)
    outs = []
    for c in range(N_CORES):
        o = res.results[c]["out"]                      # [NPAIR, FD] fp16
        o = o.reshape(CH, K, FD).transpose(1, 0, 2)    # [K, CH, FD]
        outs.append(o.reshape(1, K, H, W).astype(np.float32))
    return np.concatenate(outs, axis=0), res


def kernel(**inputs):
    out, _ = run(inputs, trace=False)
    return out
